# revision 21
# baseline (speedup 1.0000x reference)
"""Trainium2 Bass kernel for nn_Attention (B=4, N=2048, C=1024, H=16, D=64).

Tensor-parallel over heads: 8 cores x 2 heads each. Each core computes
QKV projection (transposed layout), RMSNorm+RoPE, full softmax attention
for its heads, and a partial output projection over its 128 local
channels. The 8 partial [8192, 1024] outputs are summed on the host
(the unshard step of the TP strategy -- equivalent to the all-reduce).

Key layout decisions:
  - All big matmuls use float32r (1 cycle/row at N>=512 vs 4 for f32).
  - QKV computed transposed: Q^T/K^T/V^T [128 (2 heads x 64 dim), 2048]
    per batch, so scores S^T[k, q] = (K^T-slice).T @ Q^T needs no
    transposes of activations; contraction d=64 packs the two heads into
    row-halves of the PE array (concurrent sub-array matmuls).
  - hidden_states must be transposed once ([t, c] -> [c, t]) for the QKV
    contraction over c: done with PE is_transpose tiles (optionally
    sharded across cores + AllGather).
  - softmax: no max-subtraction needed (scores bounded by RMSNorm), exp
    on ScalarE reads PSUM scores directly, writes bf16 P^T; row-sums ride
    as a 65th output row of the AV matmul via a ones-column in V_ext.
  - AV in bf16 (P^T and V_ext), f32 PSUM accumulate.
"""
import os
import sys

sys.path.insert(0, "/opt/trn_rl_repo")

import numpy as np

B, N, C, H, D = 4, 2048, 1024, 16, 64
NCORES = 8
HPC = H // NCORES          # heads per core
CL = HPC * D               # local channels (128)
T = B * N                  # 8192 tokens
CHUNK = 512
NCHUNK = T // CHUNK        # 16
EPS = 1e-6
USE_AG = os.environ.get("ATTN_USE_AG", "1") == "1"
# bisection knob: 1=transpose only, 2=+qkv, 3=+norm/rope, 4=+V, 5=+attn, 6=full
STAGES = int(os.environ.get("ATTN_STAGES", "6"))
DEBUG = os.environ.get("ATTN_DEBUG", "0") == "1"

_CACHE = {}


def _dt():
    from concourse import mybir
    return mybir


def build_graph(use_ag: bool):
    import concourse.tile as tile
    from concourse import bacc, mybir

    f32 = mybir.dt.float32
    f32r = mybir.dt.float32r
    bf16 = mybir.dt.bfloat16
    EXPF = mybir.ActivationFunctionType.Exp
    LNF = mybir.ActivationFunctionType.Ln

    nc = bacc.Bacc("TRN2", target_bir_lowering=False, debug=False,
                   num_devices=NCORES)

    # ---- external I/O ----
    if use_ag:
        xsh_e = nc.dram_tensor("xshard", [T // NCORES, C], f32r,
                               kind="ExternalInput").ap()
    else:
        x_e = nc.dram_tensor("x", [T, C], f32r, kind="ExternalInput").ap()
    wq_e = nc.dram_tensor("wqkv", [C, 3 * CL], f32r, kind="ExternalInput").ap()
    wo_e = nc.dram_tensor("wout", [CL, C], f32r, kind="ExternalInput").ap()
    cq_e = nc.dram_tensor("cosq", [CL, N], f32, kind="ExternalInput").ap()
    sq_e = nc.dram_tensor("sinq", [CL, N], f32, kind="ExternalInput").ap()
    ck_e = nc.dram_tensor("cosk", [CL, N], f32, kind="ExternalInput").ap()
    sk_e = nc.dram_tensor("sink", [CL, N], f32, kind="ExternalInput").ap()
    rotm_e = nc.dram_tensor("rotm", [128, 128], f32r, kind="ExternalInput").ap()
    blkA_e = nc.dram_tensor("blkA", [128, 2], f32r, kind="ExternalInput").ap()
    blkB_e = nc.dram_tensor("blkB", [2, 128], f32r, kind="ExternalInput").ap()
    id_e = nc.dram_tensor("ident", [128, 128], f32r, kind="ExternalInput").ap()
    out_e = nc.dram_tensor("out", [T, C], f32, kind="ExternalOutput").ap()
    if DEBUG:
        dbg_qt = nc.dram_tensor("dbg_qt", [128, N], f32,
                                kind="ExternalOutput").ap()
        dbg_kt = nc.dram_tensor("dbg_kt", [128, N], f32,
                                kind="ExternalOutput").ap()
        dbg_at = nc.dram_tensor("dbg_at", [128, N], f32,
                                kind="ExternalOutput").ap()
        dbg_atn = nc.dram_tensor("dbg_atn", [128, N], f32,
                                 kind="ExternalOutput").ap()
        dbg_rs = nc.dram_tensor("dbg_rs", [98, N], f32,
                                kind="ExternalOutput").ap()

    # ---- internal DRAM: gathered X^T, laid out [16 chunk-groups? no:
    # 8 rank-groups of 1024 tokens] -> flat [8*1024 c-rows, 1024 t-cols]
    # xt_d[g*1024 + c, tl] = X[g*1024 + tl, c]
    xt_d = nc.dram_tensor("xt", [NCORES * C, T // NCORES], f32r,
                          addr_space="Shared").ap()
    if use_ag:
        agin_d = nc.dram_tensor("agin", [C, T // NCORES], f32r).ap()

    def xt_slice(ch, ct):
        """DRAM slice of X^T for token-chunk ch (512 tokens), c-tile ct."""
        g = (ch * CHUNK) // (T // NCORES)
        tl0 = (ch * CHUNK) % (T // NCORES)
        return xt_d[g * C + ct * 128: g * C + (ct + 1) * 128, tl0: tl0 + CHUNK]

    with tile.TileContext(nc) as tc:
        from contextlib import ExitStack
        ctx = ExitStack()
        with ctx:
            # ---- constants in SBUF ----
            cpool = ctx.enter_context(tc.tile_pool(name="const", bufs=1))
            wq_sb = cpool.tile([128, 8 * 384], f32r, tag="wq")
            for ct in range(8):
                nc.sync.dma_start(wq_sb[:, ct * 384:(ct + 1) * 384],
                                  wq_e[ct * 128:(ct + 1) * 128, :])
            wo_sb = cpool.tile([128, C], f32r, tag="wo")
            nc.sync.dma_start(wo_sb[:], wo_e[:, :])
            cs_sb = cpool.tile([128, 4 * N], f32, tag="cs")
            for i, e in enumerate([cq_e, sq_e, ck_e, sk_e]):
                nc.sync.dma_start(cs_sb[:, i * N:(i + 1) * N], e[:, :])
            rotm_sb = cpool.tile([128, 128], f32r, tag="rotm")
            nc.sync.dma_start(rotm_sb[:], rotm_e[:, :])
            blkA_sb = cpool.tile([128, 2], f32r, tag="blkA")
            nc.sync.dma_start(blkA_sb[:], blkA_e[:, :])
            blkB_sb = cpool.tile([66, 128], f32r, tag="blkB")
            nc.sync.dma_start(blkB_sb[0:2, :], blkB_e[:, :])
            nc.sync.dma_start(blkB_sb[64:66, :], blkB_e[:, :])
            onesC_sb = cpool.tile([98, 64], f32, tag="onesC")
            nc.vector.memset(onesC_sb[:], 0.0)
            nc.vector.memset(onesC_sb[64:65, :], 1.0)
            nc.vector.memset(onesC_sb[96:97, :], 1.0)
            id_sb = cpool.tile([128, 128], f32r, tag="ident")
            nc.sync.dma_start(id_sb[:], id_e[:, :])
            eps_sb = cpool.tile([128, 1], f32, tag="eps")
            nc.vector.memset(eps_sb[:], EPS)

            def cosq(ch):  # ch: global chunk id; slice within batch
                n0 = (ch % 4) * CHUNK
                return cs_sb[:, 0 * N + n0: 0 * N + n0 + CHUNK]

            def sinq(ch):
                n0 = (ch % 4) * CHUNK
                return cs_sb[:, 1 * N + n0: 1 * N + n0 + CHUNK]

            def cosk(ch):
                n0 = (ch % 4) * CHUNK
                return cs_sb[:, 2 * N + n0: 2 * N + n0 + CHUNK]

            def sink(ch):
                n0 = (ch % 4) * CHUNK
                return cs_sb[:, 3 * N + n0: 3 * N + n0 + CHUNK]

            # ---- phase T: transpose X (shard or full) into xt_d ----
            with tc.tile_pool(name="xin", bufs=6) as xin_p, \
                 tc.tile_pool(name="xts", bufs=4) as xts_p, \
                 tc.tile_pool(name="tp", bufs=2, space="PSUM") as tp_p:
                my_chunks = 2 if use_ag else NCHUNK
                for ch in range(my_chunks):
                    xin = [xin_p.tile([128, C], f32r, tag="xin",
                                      name=f"xin{ch}_{s}")
                           for s in range(4)]
                    for s in range(4):
                        src = (xsh_e if use_ag else x_e)
                        r0 = ch * CHUNK + s * 128
                        nc.sync.dma_start(xin[s][:], src[r0:r0 + 128, :])
                    for ct in range(8):
                        ps = tp_p.tile([128, 512], f32r, tag="tp")
                        for s in range(4):
                            nc.tensor.transpose(
                                ps[:, s * 128:(s + 1) * 128],
                                xin[s][:, ct * 128:(ct + 1) * 128],
                                id_sb[:])
                        st = xts_p.tile([128, 512], f32r, tag="xts")
                        if ct % 2 == 0:
                            nc.vector.tensor_copy(st[:], ps[:])
                        else:
                            nc.scalar.copy(st[:], ps[:])
                        if use_ag:
                            nc.sync.dma_start(
                                agin_d[ct * 128:(ct + 1) * 128,
                                       ch * CHUNK:(ch + 1) * CHUNK], st[:])
                        else:
                            nc.sync.dma_start(xt_slice(ch, ct), st[:])
                if use_ag:
                    nc.sync.collective_compute(
                        "AllGather", mybir.AluOpType.bypass,
                        replica_groups=[list(range(NCORES))],
                        ins=[agin_d[:, :]], outs=[xt_d[:, :]])

            # ---- main pools ----
            xt_p = ctx.enter_context(tc.tile_pool(name="xt", bufs=10))
            qkvt_p = ctx.enter_context(tc.tile_pool(name="qkvt", bufs=2))
            vt_p = ctx.enter_context(tc.tile_pool(name="vtp", bufs=1))
            sc_ps = ctx.enter_context(
                tc.tile_pool(name="scp", bufs=2, space="PSUM"))
            av_ps = ctx.enter_context(
                tc.tile_pool(name="avp", bufs=2, space="PSUM"))
            mi_ps = ctx.enter_context(
                tc.tile_pool(name="mip", bufs=2, space="PSUM"))
            os_p = ctx.enter_context(tc.tile_pool(name="osp", bufs=3))
            q2_p = ctx.enter_context(tc.tile_pool(name="q2", bufs=2))
            rp_p = ctx.enter_context(tc.tile_pool(name="rp", bufs=4))
            ln_p = ctx.enter_context(tc.tile_pool(name="lns", bufs=1))
            vx_p = ctx.enter_context(tc.tile_pool(name="vx", bufs=2))
            pt_p = ctx.enter_context(tc.tile_pool(name="pt", bufs=6))
            at_p = ctx.enter_context(tc.tile_pool(name="at", bufs=2))
            rs_p = ctx.enter_context(tc.tile_pool(name="rs", bufs=1))

            for b in range(B if STAGES >= 2 else 0):
                qt_b = qkvt_p.tile([128, N], f32r, tag="qt")
                kt_b = qkvt_p.tile([128, N], f32r, tag="kt")
                vt_b = vt_p.tile([128, N], f32r, tag="vt")
                # packed norm-scale tile: rows 0-1 sqq, 32-33 lnq,
                # rows 64-65 sqk, 96-97 lnk
                na_b = ln_p.tile([98, N], f32r, tag="na")

                # --- QKV^T + sumsq ---
                for cl in range(4):
                    ch = b * 4 + cl
                    sl = slice(cl * CHUNK, (cl + 1) * CHUNK)
                    xts = []
                    for ct in range(8):
                        t = xt_p.tile([128, CHUNK], f32r, tag="xt",
                                      name=f"xt{ch}_{ct}")
                        nc.sync.dma_start(t[:], xt_slice(ch, ct))
                        xts.append(t)
                    for ot, dst in enumerate([qt_b, kt_b, vt_b]):
                        ps = mi_ps.tile([128, 512], f32, tag="mi")
                        for ct in range(8):
                            nc.tensor.matmul(
                                ps[:],
                                wq_sb[:, ct * 384 + ot * 128:
                                      ct * 384 + (ot + 1) * 128],
                                xts[ct][:],
                                start=(ct == 0), stop=(ct == 7))
                        nc.vector.tensor_copy(dst[:, sl], ps[:])
                    if STAGES < 3:
                        continue
                    # sumsq for q and k of this chunk
                    for src, ln0 in ((qt_b, 32), (kt_b, 96)):
                        q2t = q2_p.tile([128, CHUNK], f32r, tag="q2",
                                        name=f"q2_{ch}_{ln0}")
                        nc.vector.tensor_mul(q2t[:], src[:, sl], src[:, sl])
                        ss = mi_ps.tile([128, 512], f32, tag="mi",
                                        name=f"ss{ch}_{ln0}")
                        nc.tensor.matmul(ss[0:2, :], blkA_sb[:],
                                         q2t[:],
                                         start=True, stop=True)
                        nc.scalar.activation(na_b[ln0:ln0 + 2, sl],
                                             ss[0:2, :], LNF,
                                             scale=1.0 / D,
                                             bias=eps_sb[0:2, :])
                if STAGES < 4:
                    continue
                # rms scale = exp(-0.5 * ln(mean sq + eps))
                nc.scalar.activation(na_b[0:2, :], na_b[32:34, :], EXPF,
                                     scale=-0.5)
                nc.scalar.activation(na_b[64:66, :], na_b[96:98, :], EXPF,
                                     scale=-0.5)

                # --- normalize + rope ---
                for cl in range(4):
                    ch = b * 4 + cl
                    sl = slice(cl * CHUNK, (cl + 1) * CHUNK)
                    for tnsr, s0, cosf, sinf in (
                            (qt_b, 0, cosq, sinq),
                            (kt_b, 64, cosk, sink)):
                        bc = mi_ps.tile([128, 512], f32, tag="mi",
                                        name=f"bc{ch}_{s0}")
                        nc.tensor.matmul(bc[:],
                                         blkB_sb[s0:s0 + 2, :],
                                         na_b[s0:s0 + 2, sl],
                                         start=True, stop=True)
                        nc.vector.tensor_mul(tnsr[:, sl], tnsr[:, sl], bc[:])
                        rot = mi_ps.tile([128, 512], f32, tag="mi")
                        nc.tensor.matmul(rot[:], rotm_sb[:],
                                         tnsr[:, sl],
                                         start=True, stop=True)
                        t1 = rp_p.tile([128, CHUNK], f32, tag="rp")
                        nc.vector.tensor_mul(t1[:], rot[:], sinf(ch))
                        t2 = rp_p.tile([128, CHUNK], f32, tag="rp")
                        nc.vector.tensor_mul(t2[:], tnsr[:, sl], cosf(ch))
                        nc.vector.tensor_add(tnsr[:, sl], t1[:], t2[:])

                if DEBUG and b == 0:
                    nc.sync.dma_start(dbg_qt[:, :], qt_b[:].bitcast(f32))
                    nc.sync.dma_start(dbg_kt[:, :], kt_b[:].bitcast(f32))
                if STAGES < 5:
                    continue
                # --- V_ext: transpose V^T -> [t, d] per head + ones col ---
                vx_b = vx_p.tile([128, HPC * 16 * 65], bf16, tag="vx")
                # ones columns: view [128, h, tt, 65], col 64 = 1.0
                nc.vector.memset(
                    vx_b[:].rearrange("p (h t e) -> p h t e", h=HPC, t=16)
                    [:, :, :, 64:65], 1.0)
                for tg in range(4):
                    vps = av_ps.tile([128, 512], f32r, tag="av")
                    for j in range(4):
                        tt = tg * 4 + j
                        # full-square transpose: [ (h,d), t ] -> [ t, (h,d) ]
                        nc.tensor.transpose(
                            vps[:, j * 128:(j + 1) * 128],
                            vt_b[:, tt * 128:(tt + 1) * 128],
                            id_sb[:])
                    # copy [128, (j 4)(h 2)(d 64)] -> vx[128, (h 2)(tt)(65)]
                    src = vps[:].rearrange("p (j h d) -> p h j d", j=4, h=HPC)
                    dst = vx_b[:].rearrange("p (h t e) -> p h t e",
                                            h=HPC, t=16)[:, :, tg * 4:(tg + 1) * 4,
                                                         0:64]
                    nc.vector.tensor_copy(dst, src)

                if STAGES < 6:
                    continue
                # --- attention ---
                at_b = at_p.tile([128, N], f32r, tag="at")
                # rows 64/96: raw row-sums (h0/h1); rows 0/32: reciprocals
                rsrr_b = rs_p.tile([98, N], f32, tag="rsrr")
                for cl in range(4):
                    ch = b * 4 + cl
                    qsl = slice(cl * CHUNK, (cl + 1) * CHUNK)
                    avs = [av_ps.tile([128, 512], f32, tag="av",
                                      name=f"av{b}_{cl}_{h}")
                           for h in range(HPC)]
                    for grp in range(8):
                        scs = [sc_ps.tile([128, 1024], f32, tag="sc",
                                          name=f"sc{b}_{cl}_{grp}_{h}")
                               for h in range(HPC)]
                        for kk in range(2):
                            ki = grp * 2 + kk
                            for h in range(HPC):
                                nc.tensor.matmul(
                                    scs[h][:, kk * 512:(kk + 1) * 512],
                                    kt_b[h * 64:(h + 1) * 64,
                                         ki * 128:(ki + 1) * 128],
                                    qt_b[h * 64:(h + 1) * 64, qsl],
                                    start=True, stop=True)
                        pts = []
                        for h in range(HPC):
                            pt = pt_p.tile([128, 1024], bf16, tag="pt",
                                           name=f"pt{b}_{cl}_{grp}_{h}")
                            nc.scalar.activation(pt[:], scs[h][:], EXPF,
                                                 scale=float(D) ** -0.5)
                            pts.append(pt)
                        for kk in range(2):
                            ki = grp * 2 + kk
                            for h in range(HPC):
                                v0 = (h * 16 + ki) * 65
                                nc.tensor.matmul(
                                    avs[h][0:65, :],
                                    vx_b[:, v0:v0 + 65],
                                    pts[h][:, kk * 512:(kk + 1) * 512],
                                    start=(ki == 0), stop=(ki == 15))
                    for h in range(HPC):
                        nc.vector.tensor_copy(at_b[h * 64:(h + 1) * 64, qsl],
                                              avs[h][0:64, :])
                        nc.vector.tensor_copy(
                            rsrr_b[64 + 32 * h:65 + 32 * h, qsl],
                            avs[h][64:65, :])
                if DEBUG and b == 0:
                    nc.sync.dma_start(dbg_at[:, :], at_b[:].bitcast(f32))
                    nc.sync.dma_start(dbg_rs[:, :], rsrr_b[:])
                # reciprocal of row sums via exp(-ln): rows 64/96 in place
                nc.scalar.activation(rsrr_b[32:33, :], rsrr_b[64:65, :], LNF)
                nc.scalar.activation(rsrr_b[0:1, :], rsrr_b[96:97, :], LNF)
                nc.scalar.activation(rsrr_b[64:65, :], rsrr_b[32:33, :], EXPF,
                                     scale=-1.0)
                nc.scalar.activation(rsrr_b[96:97, :], rsrr_b[0:1, :], EXPF,
                                     scale=-1.0)
                # normalize A^T by reciprocal row sums
                for cl in range(4):
                    qsl = slice(cl * CHUNK, (cl + 1) * CHUNK)
                    bc = av_ps.tile([128, 512], f32, tag="av",
                                    name=f"rbc{b}_{cl}")
                    for h in range(HPC):
                        nc.tensor.matmul(
                            bc[h * 64:(h + 1) * 64, :],
                            onesC_sb[64 + 32 * h:65 + 32 * h, :],
                            rsrr_b[64 + 32 * h:65 + 32 * h, qsl],
                            start=True, stop=True,
                            tile_position=(64 + 32 * h, h * 64))
                    nc.vector.tensor_mul(at_b[:, qsl], at_b[:, qsl], bc[:])

                if DEBUG and b == 0:
                    nc.sync.dma_start(dbg_atn[:, :], at_b[:].bitcast(f32))
                if STAGES < 7:
                    continue
                # --- output projection (partial over local channels) ---
                dmae = [nc.sync, nc.gpsimd]
                for tt in range(16):
                    ost = os_p.tile([128, 1024], f32, tag="os",
                                    name=f"os{b}_{tt}")
                    for oh in range(2):
                        op = mi_ps.tile([128, 512], f32, tag="mi",
                                        name=f"op{b}_{tt}_{oh}")
                        nc.tensor.matmul(
                            op[:],
                            at_b[:, tt * 128:(tt + 1) * 128],
                            wo_sb[:, oh * 512:(oh + 1) * 512],
                            start=True, stop=True)
                        if (tt * 2 + oh) % 2 == 0:
                            nc.vector.tensor_copy(
                                ost[:, oh * 512:(oh + 1) * 512], op[:])
                        else:
                            nc.scalar.copy(
                                ost[:, oh * 512:(oh + 1) * 512], op[:])
                    dmae[tt % 2].dma_start(
                        out_e[b * N + tt * 128: b * N + (tt + 1) * 128, :],
                        ost[:])

    nc.compile()
    return nc


def _round_f32r(a):
    """Round to a bf16-pair representable value (conservative fp32r)."""
    import ml_dtypes
    a = np.asarray(a, dtype=np.float32)
    hi = a.astype(ml_dtypes.bfloat16).astype(np.float32)
    lo = (a - hi).astype(ml_dtypes.bfloat16).astype(np.float32)
    return hi + lo


def prep_in_maps(hidden_states, cos, sin, qkv_w, out_w, norm_q_w, norm_k_w,
                 use_ag: bool):
    x = np.ascontiguousarray(_round_f32r(np.asarray(
        hidden_states, dtype=np.float32).reshape(T, C)))
    cos = np.asarray(cos, dtype=np.float32)
    sin = np.asarray(sin, dtype=np.float32)
    qkv_w = np.asarray(qkv_w, dtype=np.float32)
    out_w = np.asarray(out_w, dtype=np.float32)
    wq = np.asarray(norm_q_w, dtype=np.float32)
    wk = np.asarray(norm_k_w, dtype=np.float32)

    rotm = np.zeros((128, 128), dtype=np.float32)
    for hb in range(2):
        o = hb * 64
        for i in range(32):
            rotm[o + i + 32, o + i] = -1.0   # rot[i] = -x[i+32]
            rotm[o + i, o + i + 32] = 1.0    # rot[i+32] = x[i]
    blkA = np.zeros((128, 2), dtype=np.float32)
    blkA[0:64, 0] = 1.0
    blkA[64:128, 1] = 1.0
    blkB = np.zeros((2, 128), dtype=np.float32)
    blkB[0, 0:64] = 1.0
    blkB[1, 64:128] = 1.0
    ident = np.eye(128, dtype=np.float32)

    perm = (np.arange(D) + 32) % D          # rotate-half pair index
    cosq1 = cos.T * wq[:, None]             # [64, N]
    sinq1 = sin.T * wq[perm][:, None]
    cosk1 = cos.T * wk[:, None]
    sink1 = sin.T * wk[perm][:, None]
    cosq = np.ascontiguousarray(np.concatenate([cosq1, cosq1], 0))
    sinq = np.ascontiguousarray(np.concatenate([sinq1, sinq1], 0))
    cosk = np.ascontiguousarray(np.concatenate([cosk1, cosk1], 0))
    sink = np.ascontiguousarray(np.concatenate([sink1, sink1], 0))

    in_maps = []
    for i in range(NCORES):
        hsel = slice(2 * i * D, (2 * i + 2) * D)
        rows = np.r_[np.arange(hsel.start, hsel.stop),
                     C + np.arange(hsel.start, hsel.stop),
                     2 * C + np.arange(hsel.start, hsel.stop)]
        wqkv = np.ascontiguousarray(_round_f32r(qkv_w[rows, :].T))
        wout = np.ascontiguousarray(_round_f32r(out_w[:, hsel].T))
        m = {
            "wqkv": wqkv, "wout": wout,
            "cosq": cosq, "sinq": sinq, "cosk": cosk, "sink": sink,
            "rotm": rotm, "blkA": blkA, "blkB": blkB, "ident": ident,
        }
        if use_ag:
            m["xshard"] = np.ascontiguousarray(
                x[i * (T // NCORES):(i + 1) * (T // NCORES), :])
        else:
            m["x"] = x
        in_maps.append(m)
    return in_maps


def get_nc(use_ag=USE_AG):
    key = ("nc", use_ag, STAGES)
    if key not in _CACHE:
        _CACHE[key] = build_graph(use_ag)
    return _CACHE[key]


def kernel(hidden_states, cos, sin, qkv_w, out_w, norm_q_w, norm_k_w):
    from concourse.bass_utils import run_bass_kernel_spmd
    nc = get_nc()
    in_maps = prep_in_maps(hidden_states, cos, sin, qkv_w, out_w,
                           norm_q_w, norm_k_w, USE_AG)
    res = run_bass_kernel_spmd(nc, in_maps, list(range(NCORES)))
    out = np.zeros((T, C), dtype=np.float32)
    for i in range(NCORES):
        out += res.results[i]["out"]
    return out.reshape(B, N, C)
